# revision 81
# baseline (speedup 1.0000x reference)
"""CrossMambaFusion Trainium2 kernel — 8-core SPMD via bass/Tile.

Sharding (hardcoded for B=2, C=256, H=W=64, Di=512, N=16, R=32, K=4):
  core c -> batch b = c//4, d_inner slice q = c%4 (128 channels), group
  rank r = c%4.  On-device layout is feature-major [features, tokens].
  Per core: full-L front (dec/enc proj, gating, in_proj, causal conv via
  PE diagonal matmuls, x_proj, dt), then an L-major selective scan in 4
  chunks of 1024 tokens (16 states on DVE tensor_tensor_scan, ysum/uD
  accumulated on PE via identity matmuls into PSUM), per-chunk partial
  m_out (own 128 d rows), chunked ReduceScatter over the 4-core batch
  group, and a per-chunk tail (gate via tanh trick, out_proj, residual,
  layernorm with Rsqrt).  Core c owns tokens {q*1024 + r*256 .. +256}
  for q=0..3 and returns res [C, 1024] on those tokens.

  Engine budget: DVE keeps only scans + elementwise muls (GpSimd is
  poison: SBUF port contention halves/decimates DVE), PE absorbs conv,
  ysum accumulation and all GEMMs, ACT does activations (act-table
  thrash avoided: tanh instead of sigmoid for the out-gate so the scan
  phase stays in the exp table; Rsqrt kills sqrt+reciprocal in LN).
"""
import numpy as np
import ml_dtypes

bf16 = ml_dtypes.bfloat16

B, C, Hh, Ww = 2, 256, 64, 64
L = Hh * Ww
Di, N, R, KC = 512, 16, 32, 4
DQ = 128
NCORES = 8
LCH = 1024          # scan/tail chunk
NCH = L // LCH      # 4 chunks
LSUB = LCH // 4     # 256: my token sub-block per chunk
LC = 512            # matmul free-dim tile
NL = L // LC

_cache = {}


def _build():
    import concourse.bass as bass
    import concourse.mybir as mybir
    import concourse.tile as tile
    from concourse import bacc

    fp32 = mybir.dt.float32
    bfl = mybir.dt.bfloat16
    AF = mybir.ActivationFunctionType
    OP = mybir.AluOpType
    ts = bass.ts

    nc = bacc.Bacc("TRN2", target_bir_lowering=False, num_devices=NCORES)

    def din(name, shape, dt=fp32):
        return nc.declare_dram_parameter(name, list(shape), dt, isOutput=False)

    dec_bf = din("dec_bf", (C, L), bfl)
    enc_bf = din("enc_bf", (C, L), bfl)
    dec_f32m = din("dec_f32m", (C, LCH), fp32)     # residual at my tokens
    dec_gm = din("dec_gm", (C, LCH), bfl)          # dec at my tokens (gate proj)
    w_dec_x = din("w_dec_x", (C, Di), bfl)
    w_dec_g = din("w_dec_g", (C, Di), bfl)
    b_dec_x = din("b_dec_x", (Di, 1))
    b_dec_gh = din("b_dec_gh", (Di, 1))            # dec_b gate half (b/2)
    w_enc = din("w_enc", (C, Di), bfl)
    b_enc = din("b_enc", (Di, 1))
    w_in_x = din("w_in_x", (Di, Di), bfl)          # columns permuted (own slice first)
    b_in_x = din("b_in_x", (Di, 1))                # permuted
    w_in_z = din("w_in_z", (Di, DQ), bfl)
    b_in_z = din("b_in_z", (DQ, 1))
    conv_w4 = din("conv_w4", (Di, KC))             # permuted rows
    conv_b = din("conv_b", (Di, 1))                # permuted
    w_xp = din("w_xp", (Di, 2 * R), bfl)           # permuted rows
    w_dt = din("w_dt", (R, DQ), bfl)
    b_dt = din("b_dt", (DQ, 1))
    a_sl = din("a_sl", (DQ, N))
    d_col = din("d_col", (DQ, 1))
    id_mat = din("id_mat", (DQ, DQ), bfl)
    w_mo = din("w_mo", (DQ, Di), bfl)              # m_out rows for own d slice
    b_mo = din("b_mo", (Di, 1))
    w_out = din("w_out", (Di, C), bfl)             # pre-scaled by 0.5 (tanh gate)
    b_out = din("b_out", (C, 1))
    g_col = din("g_col", (C, 1))
    bln_col = din("bln_col", (C, 1))

    res_out = nc.declare_dram_parameter("res", [C, LCH], fp32, isOutput=True)

    with tile.TileContext(nc) as tc:
        import contextlib
        with contextlib.ExitStack() as stack:
            wpool = stack.enter_context(tc.tile_pool(name="weights", bufs=1))
            cpool = stack.enter_context(tc.tile_pool(name="consts", bufs=1))
            dpool = stack.enter_context(tc.tile_pool(name="drambuf", bufs=1, space="DRAM"))

            bc_rows = [dpool.tile([2 * N, LCH], bfl, name=f"bc_rows{i}", tag=f"bc_rows{i}")
                       for i in range(NCH)]           # B rows then C rows, per chunk
            rs_in = [dpool.tile([4 * Di, LSUB], bfl, name=f"rs_in{i}", tag=f"rs_in{i}")
                     for i in range(NCH)]
            rs_out = [dpool.tile([Di, LSUB], bfl, name=f"rs_out{i}", tag=f"rs_out{i}")
                      for i in range(NCH)]
            ln_rows = [dpool.tile([2, LSUB], fp32, name=f"ln_rows{i}", tag=f"ln_rows{i}")
                       for i in range(NCH)]

            # weights/consts load on the Scalar ring so the Sync ring streams
            # the dec/enc activations first (startup-critical path)
            def wload(ap, kt, m, name):
                t = wpool.tile([128, kt, m], bfl, tag=name, name=name)
                nc.scalar.dma_start(out=t[:], in_=ap.ap().rearrange("(t k) m -> k t m", k=128))
                return t

            sw_dec_x = wload(w_dec_x, 2, Di, "w_dec_x")
            sw_enc = wload(w_enc, 2, Di, "w_enc")
            sw_dec_g = wload(w_dec_g, 2, Di, "w_dec_g")

            def cload(ap, nt, name, cols=1):
                if nt == 1:
                    t = cpool.tile([128, cols], fp32, tag=name, name=name)
                    nc.scalar.dma_start(out=t[:], in_=ap.ap())
                else:
                    t = cpool.tile([128, nt, cols], fp32, tag=name, name=name)
                    nc.scalar.dma_start(out=t[:], in_=ap.ap().rearrange("(t k) o -> k t o", k=128))
                return t

            sb_dec_x = cload(b_dec_x, 4, "b_dec_x")
            sb_dec_gh = cload(b_dec_gh, 4, "b_dec_gh")
            sb_enc = cload(b_enc, 4, "b_enc")
            sb_in_x = cload(b_in_x, 4, "b_in_x")
            sb_in_z = cload(b_in_z, 1, "b_in_z")
            s_convw = cload(conv_w4, 4, "conv_w4", cols=KC)
            s_convb = cload(conv_b, 4, "conv_b")
            sb_dt = cload(b_dt, 1, "b_dt")
            s_a = cload(a_sl, 1, "a_sl", cols=N)
            s_d = cload(d_col, 1, "d_col")
            sb_mo = cload(b_mo, 4, "b_mo")
            sb_out = cload(b_out, 2, "b_out")
            s_g = cload(g_col, 2, "g_col")
            s_bln = cload(bln_col, 2, "bln_col")

            sw_in_x = wload(w_in_x, 4, Di, "w_in_x")
            sw_in_z = wload(w_in_z, 4, DQ, "w_in_z")
            sw_xp = wload(w_xp, 4, 2 * R, "w_xp")
            sw_mo = wload(w_mo, 1, Di, "w_mo")
            sw_out = wload(w_out, 4, C, "w_out")
            sw_id = wload(id_mat, 1, DQ, "id_mat")
            sw_dt = wpool.tile([R, DQ], bfl)
            nc.scalar.dma_start(out=sw_dt[:], in_=w_dt.ap())

            # persistent scan inputs (full L) + pretail products
            spool0 = stack.enter_context(tc.tile_pool(name="scanin", bufs=1))
            s_dt = spool0.tile([128, L], bfl)
            s_dtu = spool0.tile([128, L], bfl)
            s_uD = spool0.tile([128, L], bfl)
            s_uown = spool0.tile([128, L], bfl)     # u for own d slice (m=0)
            s_siluz = spool0.tile([128, L], bfl)
            s_tg = spool0.tile([128, 4, LCH], bfl)     # tanh(gate/2) at my tokens
            decm_r = dec_f32m.ap().rearrange("(t k) l -> k t l", k=128)

            # ---- pretail: out-gate = 1 + tanh((dec_gate)/2) at my tokens.
            # Emitted FIRST: dec_gm is a small separate input, so PE/ACT start
            # immediately while the big dec/enc activations stream in.
            with tc.tile_pool(name="pretail", bufs=1) as ptpool, \
                 tc.tile_pool(name="psg", bufs=2, space="PSUM") as psg:
                s_decq = ptpool.tile([128, 2, LCH], bfl)
                nc.sync.dma_start(out=s_decq[:],
                                  in_=dec_gm.ap().rearrange("(t k) l -> k t l", k=128))
                for lc in range(LCH // LC):
                    ls = ts(lc, LC)
                    for m in range(4):
                        ps_gt = psg.tile([128, LC], fp32, tag="ps_gt", name="ps_gt")
                        for t in range(2):
                            nc.tensor.matmul(ps_gt[:], sw_dec_g[:, t, ts(m, 128)],
                                             s_decq[:, t, ls], start=(t == 0), stop=(t == 1))
                        nc.scalar.activation(s_tg[:, m, ls], ps_gt[:], AF.Tanh,
                                             scale=0.5, bias=sb_dec_gh[:, m, :])

            with tc.tile_pool(name="ph12", bufs=1) as ppool:
                s_comb = ppool.tile([128, 4, L], bfl)
                s_u = ppool.tile([128, 3, L], bfl)   # u for m=1..3 (m=0 -> s_uown)
                s_dec = ppool.tile([128, 2, L], bfl)
                s_enc = ppool.tile([128, 2, L], bfl)
                dec_r = dec_bf.ap().rearrange("(t k) l -> k t l", k=128)
                enc_r = enc_bf.ap().rearrange("(t k) l -> k t l", k=128)
                for pc in range(4):
                    pl = ts(pc, L // 4)
                    nc.sync.dma_start(out=s_dec[:, :, pl], in_=dec_r[:, :, pl])
                    nc.sync.dma_start(out=s_enc[:, :, pl], in_=enc_r[:, :, pl])

                # ---- pretail: out-gate tanh at my tokens (overlaps front) ----
                # my tokens: chunk q, cols [r*256, (r+1)*256) -> input cols are
                # per-core: host passes dec_bf full; my col slices are static
                # per core?? no: r varies per core. Use separate input instead.
                # (handled below via dec_gm input)

                # ---- phase 1: combined = dec_x*sig(enc_p) + enc_p ----
                with tc.tile_pool(name="ph1c", bufs=2) as f1c, \
                     tc.tile_pool(name="ps1", bufs=1, space="PSUM") as ps1:
                    # weight-stationary: per (m, 4-lc block), 4 ldweights total
                    for m in range(4):
                        for blk in range(2):
                            pdx = []
                            pep = []
                            for j in range(4):
                                t0 = ps1.tile([128, LC], fp32, tag=f"ps_dx{j}", name="pdx")
                                pdx.append(t0)
                                t1 = ps1.tile([128, LC], fp32, tag=f"ps_ep{j}", name="pep")
                                pep.append(t1)
                            for t in range(2):
                                for j in range(4):
                                    lc = blk * 4 + j
                                    nc.tensor.matmul(pdx[j][:], sw_dec_x[:, t, ts(m, 128)],
                                                     s_dec[:, t, ts(lc, LC)],
                                                     start=(t == 0), stop=(t == 1))
                            for t in range(2):
                                for j in range(4):
                                    lc = blk * 4 + j
                                    nc.tensor.matmul(pep[j][:], sw_enc[:, t, ts(m, 128)],
                                                     s_enc[:, t, ts(lc, LC)],
                                                     start=(t == 0), stop=(t == 1))
                            for j in range(4):
                                lc = blk * 4 + j
                                ls = ts(lc, LC)
                                sg = f1c.tile([128, LC], bfl, tag="sg", name="sg")
                                nc.scalar.activation(sg[:], pep[j][:], AF.Sigmoid,
                                                     bias=sb_enc[:, m, :])
                                tm = f1c.tile([128, LC], bfl, tag="tm", name="tm")
                                nc.vector.scalar_tensor_tensor(tm[:], pdx[j][:],
                                                               sb_dec_x[:, m, :], sg[:],
                                                               OP.add, OP.mult)
                                nc.vector.scalar_tensor_tensor(s_comb[:, m, ls], pep[j][:],
                                                               sb_enc[:, m, :], tm[:],
                                                               OP.add, OP.add)

                # ---- pretail: gate tanh for my tokens from s_dec slices ----
                # my cols per chunk q: q*1024 + r*256 .. +256. r is per-core so
                # host passes a per-core gathered bf16 copy of dec at my tokens.
                # (declared below, loaded into s_decq)

                # ---- phase 2: in_proj, conv (PE diag), silu, x_proj, dt ----
                with tc.tile_pool(name="ph2c", bufs=2) as m2c, \
                     tc.tile_pool(name="ps2x", bufs=1, space="PSUM") as ps2x, \
                     tc.tile_pool(name="ps2", bufs=2, space="PSUM") as ps2:
                    s_xm = ppool.tile([128, 4, 3 + L], bfl)
                    for m in range(4):
                        nc.vector.memset(s_xm[:, m, 0:3], 0.0)
                    # in_proj + z: weight-stationary over 4-lc blocks
                    for m in range(5):      # m=4 is the z projection
                        for blk in range(2):
                            pxm = []
                            for j in range(4):
                                t0 = ps2x.tile([128, LC], fp32, tag=f"pmm{j}", name="pxm")
                                pxm.append(t0)
                            for t in range(4):
                                for j in range(4):
                                    lc = blk * 4 + j
                                    if m < 4:
                                        nc.tensor.matmul(pxm[j][:], sw_in_x[:, t, ts(m, 128)],
                                                         s_comb[:, t, ts(lc, LC)],
                                                         start=(t == 0), stop=(t == 3))
                                    else:
                                        nc.tensor.matmul(pxm[j][:], sw_in_z[:, t, :],
                                                         s_comb[:, t, ts(lc, LC)],
                                                         start=(t == 0), stop=(t == 3))
                            for j in range(4):
                                lc = blk * 4 + j
                                if m < 4:
                                    nc.scalar.activation(
                                        s_xm[:, m, 3 + lc * LC:3 + (lc + 1) * LC],
                                        pxm[j][:], AF.Identity, bias=sb_in_x[:, m, :])
                                else:
                                    nc.scalar.activation(s_siluz[:, ts(lc, LC)],
                                                         pxm[j][:], AF.Silu,
                                                         bias=sb_in_z[:, 0:1])

                    # causal depthwise conv: m=0,1 on DVE (stt chains),
                    # m=2,3 on PE (diagonal matmuls) — balances front engines
                    # piece-outer so x_proj lc-chunks unblock as early as possible
                    for piece in range(4):
                        for m in range(4):
                            o = piece * (L // 4)
                            acc = m2c.tile([128, L // 4], fp32, tag=f"acc{m % 2}",
                                           name="acc")
                            nc.vector.tensor_scalar(acc[:], s_xm[:, m, o:o + L // 4],
                                                    s_convw[:, m, 0:1], None, OP.mult)
                            for k in range(1, KC):
                                nc.vector.scalar_tensor_tensor(
                                    acc[:], s_xm[:, m, o + k:o + k + L // 4],
                                    s_convw[:, m, k:k + 1], acc[:], OP.mult, OP.add)
                            udst = (s_uown[:, o:o + L // 4] if m == 0
                                    else s_u[:, m - 1, o:o + L // 4])
                            nc.scalar.activation(udst, acc[:], AF.Silu,
                                                 bias=s_convb[:, m, :])

                    # x_proj + B/C row spill + dt (softplus)
                    for lc in range(NL):
                        ls = ts(lc, LC)
                        ps_xd = ps2.tile([128, LC], fp32, tag="xddt", name="ps_xd")
                        for t in range(4):
                            urhs = s_uown[:, ls] if t == 0 else s_u[:, t - 1, ls]
                            nc.tensor.matmul(ps_xd[0:64, :], sw_xp[:, t, :], urhs,
                                             start=(t == 0), stop=(t == 3))
                        dtin = m2c.tile([R, LC], bfl, tag="dtin", name="dtin")
                        nc.scalar.activation(dtin[:], ps_xd[0:R, :], AF.Copy)
                        bcl = m2c.tile([64, LC], bfl, tag="bcl", name="bcl")
                        nc.scalar.activation(bcl[32:64, :], ps_xd[32:64, :], AF.Copy)
                        nc.sync.dma_start(out=bc_rows[lc // 2][:, ts(lc % 2, LC)],
                                          in_=bcl[32:64, :])
                        ps_dt = ps2.tile([128, LC], fp32, tag="xddt", name="ps_dt")
                        nc.tensor.matmul(ps_dt[:], sw_dt[:, :], dtin[:],
                                         start=True, stop=True)
                        # softplus(x) = ln(1 + exp(x)); x = psum + dt_b (x << 88)
                        ex = m2c.tile([128, LC], fp32, tag="sp2", name="ex")
                        nc.scalar.activation(ex[:], ps_dt[:], AF.Exp, bias=sb_dt[:, 0:1])
                        nc.scalar.activation(s_dt[:, ls], ex[:], AF.Ln, bias=1.0)
                        # dtu = dt * u_own ; uD = u_own * D per scan chunk as
                        # soon as its dt slice exists
                        if lc % 2 == 1:
                            hs = ts(lc // 2, LCH)
                            nc.vector.tensor_tensor(s_dtu[:, hs], s_dt[:, hs],
                                                    s_uown[:, hs], OP.mult)
                            nc.vector.tensor_scalar(s_uD[:, hs], s_uown[:, hs],
                                                    s_d[:, 0:1], None, OP.mult)

            # ---- scan (L-major, 4 chunks) + per-chunk mo/RS/tail ----
            spool = stack.enter_context(tc.tile_pool(name="scan", bufs=2))
            bpool = stack.enter_context(tc.tile_pool(name="scanb", bufs=1))
            s1pool = stack.enter_context(tc.tile_pool(name="scan1", bufs=1))
            tpool = stack.enter_context(tc.tile_pool(name="tail", bufs=2))
            psy = stack.enter_context(tc.tile_pool(name="psy", bufs=2, space="PSUM"))
            psmo = stack.enter_context(tc.tile_pool(name="psmo", bufs=1, space="PSUM"))
            psln = stack.enter_context(tc.tile_pool(name="psln", bufs=1, space="PSUM"))
            s_hlast = s1pool.tile([128, N], bfl)
            ones = s1pool.tile([128, 1], fp32, tag="ones", name="ones")
            nc.vector.memset(ones[:], 1.0)
            ones_row = s1pool.tile([1, 128], fp32, tag="ones_row", name="ones_row")
            nc.vector.memset(ones_row[:], 1.0)
            eps = s1pool.tile([1, 1], fp32, tag="eps", name="eps")
            nc.vector.memset(eps[:], 1e-5)

            def emit_scan_mo(ch):
                cs = ts(ch, LCH)
                co = ch * LCH
                ysum = [psy.tile([128, LC], fp32, tag=f"ysum{sl}", name=f"ysum{sl}")
                        for sl in range(2)]
                # uD as first accumulation step
                for sl in range(2):
                    nc.tensor.matmul(ysum[sl][:], sw_id[:, 0, :],
                                     s_uD[:, co + sl * LC:co + (sl + 1) * LC],
                                     start=True, stop=False)
                bca = bc_rows[ch][0:1, 0:1]
                for n in range(N):
                    w = n % 2
                    # one broadcast DMA per state: rows n (B) and N+n (C)
                    bc2 = bpool.tile([128, 2, LCH], bfl, tag=f"bc{n % 12}", name="bc2")
                    nc.sync.dma_start(out=bc2[:], in_=bass.AP(
                        tensor=bca.tensor, offset=bca.offset + n * LCH,
                        ap=[[0, 128], [N * LCH, 2], [1, LCH]]))
                    a = spool.tile([128, LCH], bfl, tag=f"a{w}", name=f"a{w}")
                    nc.scalar.activation(a[:], s_dt[:, cs], AF.Exp,
                                         scale=s_a[:, n:n + 1])
                    bt = spool.tile([128, LCH], bfl, tag=f"b{w}", name=f"b{w}")
                    nc.vector.tensor_tensor(bt[:], s_dtu[:, cs], bc2[:, 0, :], OP.mult)
                    h = spool.tile([128, LCH], bfl, tag=f"h{w}", name=f"h{w}")
                    init = 0.0 if ch == 0 else s_hlast[:, n:n + 1]
                    nc.vector.tensor_tensor_scan(h[:], a[:], bt[:], init,
                                                 OP.mult, OP.add)
                    if ch < NCH - 1:
                        nc.vector.tensor_copy(s_hlast[:, n:n + 1], h[:, LCH - 1:LCH])
                    g = spool.tile([128, LCH], bfl, tag=f"g{w}", name=f"g{w}")
                    nc.vector.tensor_tensor(g[:], h[:], bc2[:, 1, :], OP.mult)
                    for sl in range(2):
                        nc.tensor.matmul(ysum[sl][:], sw_id[:, 0, :],
                                         g[:, ts(sl, LC)],
                                         start=False, stop=(n == N - 1))
                # y = ysum * siluz
                yc = spool.tile([128, LCH], bfl, tag="yc", name="yc")
                for sl in range(2):
                    nc.scalar.activation(yc[:, ts(sl, LC)], ysum[sl][:], AF.Copy)
                y = spool.tile([128, LCH], bfl, tag="y", name="y")
                nc.vector.tensor_tensor(y[:], yc[:], s_siluz[:, cs], OP.mult)
                # partial m_out (own 128 d rows): [512 m, LCH]
                s_mo = tpool.tile([128, 4, LCH], bfl, tag="s_mo", name="s_mo")
                for mm in range(4):
                    ps_mo = psmo.tile([128, LCH], fp32, tag="mo", name="ps_mo")
                    for sl in range(2):
                        nc.tensor.matmul(ps_mo[:, ts(sl, LC)], sw_mo[:, 0, ts(mm, 128)],
                                         y[:, ts(sl, LC)], start=True, stop=True)
                    nc.scalar.activation(s_mo[:, mm, :], ps_mo[:], AF.Copy)
                # spill to DRAM rs_in: block r = cols [r*256, +256), rows (k t)
                for r in range(4):
                    nc.gpsimd.dma_start(
                        out=rs_in[ch][r * Di:(r + 1) * Di, :].rearrange(
                            "(k t) l -> k t l", k=128),
                        in_=s_mo[:, :, ts(r, LSUB)])
                nc.gpsimd.collective_compute(
                    "ReduceScatter", mybir.AluOpType.add,
                    replica_groups=[[0, 1, 2, 3], [4, 5, 6, 7]],
                    ins=[rs_in[ch][:, :]], outs=[rs_out[ch][:, :]],
                )

            def emit_tail(ch):
                # ---- tail for my LSUB tokens of this chunk ----
                ms = ts(ch, LSUB)   # my token cols within [0, LCH) output space
                s_spr = tpool.tile([128, 4, LSUB], bfl, tag="s_spr", name="s_spr")
                nc.scalar.dma_start(out=s_spr[:],
                                    in_=rs_out[ch].rearrange("(k t) l -> k t l", k=128))
                s_decm = tpool.tile([128, 2, LSUB], fp32, tag="s_decm", name="s_decm")
                nc.scalar.dma_start(out=s_decm[:], in_=decm_r[:, :, ms])
                s_gated = tpool.tile([128, 4, LSUB], bfl, tag="s_gated", name="s_gated")
                for mm in range(4):
                    sprb = tpool.tile([128, LSUB], bfl, tag="sprb", name="sprb")
                    nc.scalar.activation(sprb[:], s_spr[:, mm, :], AF.Identity,
                                         bias=sb_mo[:, mm, :])
                    # gated = sprb * (1 + tg)
                    nc.vector.scalar_tensor_tensor(s_gated[:, mm, :],
                                                   s_tg[:, mm, ms], 1.0, sprb[:],
                                                   OP.add, OP.mult)
                s_res = tpool.tile([128, 2, LSUB], fp32, tag="s_res", name="s_res")
                s_res2 = tpool.tile([128, 2, LSUB], fp32, tag="s_res2", name="s_res2")
                ps_sum = psln.tile([1, LSUB], fp32, tag="lnsum", name="ps_sum")
                ps_sq = psln.tile([1, LSUB], fp32, tag="lnsq", name="ps_sq")
                for mm in range(2):
                    ps_o = psmo.tile([128, LSUB], fp32, tag="mo", name="ps_o")
                    for t in range(4):
                        nc.tensor.matmul(ps_o[:], sw_out[:, t, ts(mm, 128)],
                                         s_gated[:, t, :], start=(t == 0), stop=(t == 3))
                    nc.vector.scalar_tensor_tensor(s_res[:, mm, :], ps_o[:],
                                                   sb_out[:, mm, :], s_decm[:, mm, :],
                                                   OP.add, OP.add)
                    nc.scalar.activation(s_res2[:, mm, :], s_res[:, mm, :], AF.Square)
                    nc.tensor.matmul(ps_sum[:], ones[:], s_res[:, mm, :],
                                     start=(mm == 0), stop=(mm == 1))
                    nc.tensor.matmul(ps_sq[:], ones[:], s_res2[:, mm, :],
                                     start=(mm == 0), stop=(mm == 1))
                mu = tpool.tile([1, LSUB], fp32, tag="mu", name="mu")
                musq = tpool.tile([1, LSUB], fp32, tag="musq", name="musq")
                var = tpool.tile([1, LSUB], fp32, tag="var", name="var")
                rstd = tpool.tile([1, LSUB], fp32, tag="rstd", name="rstd")
                nc.scalar.activation(mu[:], ps_sum[:], AF.Copy, scale=1.0 / C)
                nc.scalar.activation(musq[:], mu[:], AF.Square)
                nc.vector.scalar_tensor_tensor(var[:], ps_sq[:], 1.0 / C,
                                               musq[:], OP.mult, OP.subtract)
                sd = tpool.tile([1, LSUB], fp32, tag="sd", name="sd")
                nc.scalar.activation(sd[:], var[:], AF.Sqrt, bias=eps[:, 0:1])
                nc.vector.reciprocal(rstd[:], sd[:])
                # broadcast mu/rstd to 128 partitions via PE ones-matmul
                # (avoids a ~5us DRAM round-trip on the exposed tail path)
                mu_bc = psln.tile([128, LSUB], fp32, tag="lnsum", name="mu_bc")
                rs_bc = psln.tile([128, LSUB], fp32, tag="lnsq", name="rs_bc")
                nc.tensor.matmul(mu_bc[:], ones_row[:], mu[:], start=True, stop=True)
                nc.tensor.matmul(rs_bc[:], ones_row[:], rstd[:], start=True, stop=True)
                for mm in range(2):
                    t1 = tpool.tile([128, LSUB], fp32, tag="t1", name="t1")
                    nc.vector.tensor_tensor(t1[:], s_res[:, mm, :], mu_bc[:],
                                            OP.subtract)
                    nc.vector.tensor_tensor(t1[:], t1[:], rs_bc[:], OP.mult)
                    nc.scalar.activation(t1[:], t1[:], AF.Identity,
                                         scale=s_g[:, mm, :], bias=s_bln[:, mm, :])
                    nc.scalar.dma_start(
                        out=res_out.ap().rearrange("(t k) l -> k t l", k=128)[:, mm, ms],
                        in_=t1[:])

            # software pipeline depth 2: tail(c) emitted after scan(c+2) so the
            # DVE/PE streams never stall on RS#c
            for ch in range(NCH):
                emit_scan_mo(ch)
                if ch >= 2:
                    emit_tail(ch - 2)
            emit_tail(NCH - 2)
            emit_tail(NCH - 1)

    nc.compile()
    return nc


def _in_maps(inp):
    A = -np.exp(inp["A_log"].astype(np.float32))
    dec_T = inp["decoder_feat"].reshape(B, C, L)
    enc_T = inp["encoder_feat"].reshape(B, C, L)
    dec_T_bf = dec_T.astype(bf16)
    enc_T_bf = enc_T.astype(bf16)

    def col(x):
        return np.ascontiguousarray(np.asarray(x, np.float32).reshape(-1, 1))

    common = {
        "w_dec_x": np.ascontiguousarray(inp["dec_w"][:, :Di].astype(bf16)),
        "w_dec_g": np.ascontiguousarray(inp["dec_w"][:, Di:].astype(bf16)),
        "b_dec_x": col(inp["dec_b"][:Di]),
        "b_dec_gh": col(inp["dec_b"][Di:] * 0.5),
        "w_enc": inp["enc_w"].astype(bf16),
        "b_enc": col(inp["enc_b"]),
        "b_mo": col(inp["m_out_b"]),
        "w_out": (inp["out_w"] * 0.5).astype(bf16),
        "b_out": col(inp["out_b"]),
        "g_col": col(inp["ln_g"]),
        "bln_col": col(inp["ln_b"]),
        "id_mat": np.eye(DQ, dtype=np.float32).astype(bf16),
    }

    in_maps = []
    for c in range(NCORES):
        b, q = c // 4, c % 4
        r = c % 4
        ds = slice(q * DQ, (q + 1) * DQ)
        perm = np.r_[np.arange(q * DQ, (q + 1) * DQ),
                     np.arange(0, q * DQ), np.arange((q + 1) * DQ, Di)]
        mycols = np.concatenate(
            [np.arange(ch * LCH + r * LSUB, ch * LCH + (r + 1) * LSUB)
             for ch in range(NCH)])
        m = dict(common)
        m["dec_bf"] = dec_T_bf[b]
        m["enc_bf"] = enc_T_bf[b]
        m["dec_f32m"] = np.ascontiguousarray(dec_T[b][:, mycols].astype(np.float32))
        m["dec_gm"] = np.ascontiguousarray(dec_T_bf[b][:, mycols])
        m["w_in_x"] = np.ascontiguousarray(inp["in_w"][:, :Di][:, perm].astype(bf16))
        m["b_in_x"] = col(inp["in_b"][:Di][perm])
        m["w_in_z"] = np.ascontiguousarray(
            inp["in_w"][:, Di + q * DQ:Di + (q + 1) * DQ].astype(bf16))
        m["b_in_z"] = col(inp["in_b"][Di + q * DQ:Di + (q + 1) * DQ])
        m["conv_w4"] = np.ascontiguousarray(inp["conv_w"][perm, 0, :].astype(np.float32))
        m["conv_b"] = col(inp["conv_b"][perm])
        m["w_xp"] = np.ascontiguousarray(inp["x_proj_w"][perm, :].astype(bf16))
        m["w_dt"] = np.ascontiguousarray(inp["dt_w"][:, ds].astype(bf16))
        m["b_dt"] = col(inp["dt_b"][ds])
        m["w_mo"] = np.ascontiguousarray(inp["m_out_w"][ds, :].astype(bf16))
        m["a_sl"] = np.ascontiguousarray(A[ds])
        m["d_col"] = col(inp["D_param"][ds])
        in_maps.append(m)
    return in_maps


def kernel(**inputs):
    from concourse.bass_utils import run_bass_kernel_spmd

    inp = {k: np.asarray(v) for k, v in inputs.items()}
    if "nc" not in _cache:
        _cache["nc"] = _build()
    res = run_bass_kernel_spmd(_cache["nc"], _in_maps(inp), list(range(NCORES)))
    out = np.zeros((B, C, L), np.float32)
    for c in range(NCORES):
        b, r = c // 4, c % 4
        mycols = np.concatenate(
            [np.arange(ch * LCH + r * LSUB, ch * LCH + (r + 1) * LSUB)
             for ch in range(NCH)])
        out[b][:, mycols] = res.results[c]["res"]
    return out.reshape(B, C, Hh, Ww)


def run_traced(inp):
    from concourse.bass_utils import run_bass_kernel_spmd

    if "nc" not in _cache:
        _cache["nc"] = _build()
    return run_bass_kernel_spmd(_cache["nc"], _in_maps(inp), list(range(NCORES)),
                                trace=True)
